# revision 26
# baseline (speedup 1.0000x reference)
"""Trainium2 Bass kernel for nn_LRSVConv (low-rank spatially-varying conv).

Computes, for full inputs
    x            [8, 32, 256, 256]  f32
    conv_w       [192, 32, 3, 3]    f32   (192 = RANK(3) * C_OUT(64))
    kernel_weight[2, 256, 256]      f32
the reference:
    y   = conv2d(x, conv_w, stride 1, pad 1)      # [8, 192, 256, 256]
    y   = y.reshape(8, 3, 64, 256, 256)
    out = y[:,0] + kw[0]*y[:,1] + kw[1]*y[:,2]    # [8, 64, 256, 256]

Strategy: spatial (H) sharding across 8 cores - each core computes a band of
32 output rows for ALL batches, so the per-pixel blend weights (which are
batch-independent) are loaded/broadcast once per core and reused 8x.

The conv matmuls run in bf16 (inputs and weights quantized host-side; fp32
PSUM accumulate). Measured absmax rel err ~2.4e-3, inside the 2e-2 gate.
All matmuls use uniform (128,128) PE tiles (a2 weights zero-padded to
M=128) to avoid PE tile-config switches between matmuls.

Per core:
  - imcol tile [96, 32*258] bf16: 3 kh-shifted replicas of the padded input
    rows (partition dim = (kh, c_in)), padded W=258 so kw shifts are free-dim
    offsets and no edge handling is needed.
  - conv: per supertile (4 output rows = 1024 px, split into 2 blocks of
    512 px), per rank r and kw: one K=96, M=128, N=512 matmul per block,
    accumulating in PSUM; bc = ranks 1,2 [128, 1024], a2 = rank 0
    [128, 512] with psum rows = (block, c_out).
  - blend: m = bc * svb on DVE; m folded onto a2 via identity matmuls on
    TensorE (partition-fold is only possible on PE or DVE-with-PSUM);
    out = copy(a2) on Scalar (PSUM evacuation), then 2 output DMAs on the
    scalar engine's HW DGE queue (inputs load on sync's queue).
  - sv broadcast tiles prepared host-side ([128, 8192] f32: rows
    (block, c) x band pixels).
"""

import os

import numpy as np
import ml_dtypes

B, C_IN, C_OUT, RANK, IMG = 8, 32, 64, 3, 256
N_CORES = 8
BAND = IMG // N_CORES          # 32 output rows per core
WP = IMG + 2                   # padded width 258
ROWS_IN = BAND + 2             # input rows needed per band (with halo)
SUPER = 8                      # supertiles per (batch, band): 4 rows each
SROWS = BAND // SUPER          # 4 image rows per supertile
NBLK = 512                     # pixels per matmul block (2 image rows)

_F32 = np.float32
_BF16 = ml_dtypes.bfloat16

NB = int(os.environ.get("KERNEL_NB", str(B)))  # batches to process (debug knob)


def _build_bass():
    import concourse.mybir as mybir
    import concourse.tile as tile
    from concourse import bacc

    f32 = mybir.dt.float32
    f32r = mybir.dt.float32r
    bf16 = mybir.dt.bfloat16
    nc = bacc.Bacc("TRN2", target_bir_lowering=False, debug=False)

    xs_t = nc.dram_tensor("xs", (B, C_IN, ROWS_IN * WP), bf16, kind="ExternalInput")
    # wtbc[kw]: [96, (rank1|rank2)]; wta[kw, q]: [96, (w0|0) or (0|w0)]
    wtbc_t = nc.dram_tensor("wtbc", (3, 96, 128), bf16, kind="ExternalInput")
    wta_t = nc.dram_tensor("wta", (3, 2, 96, 128), bf16, kind="ExternalInput")
    # S12: rows 0:64 = sv1, rows 64:128 = sv2; cols = (supertile, block, j)
    svb_t = nc.dram_tensor("svb", (128, SUPER * 2 * NBLK), bf16, kind="ExternalInput")
    # kernel-native layout: (b, t, c, (q r w)); host untangles it
    out_t = nc.dram_tensor("out", (B, SUPER, 64, 2 * NBLK), bf16, kind="ExternalOutput")

    xs = xs_t.ap()
    out_r = out_t.ap()

    with tile.TileContext(nc) as tc:
        with (
            tc.tile_pool(name="const", bufs=1) as cpool,
            tc.tile_pool(name="imcol", bufs=2) as ipool,
            tc.tile_pool(name="psum", bufs=2, space="PSUM") as ppool,
            tc.tile_pool(name="warm", bufs=1, space="PSUM") as wpool,
            tc.tile_pool(name="tmp", bufs=4) as tpool,
            tc.tile_pool(name="outp", bufs=6) as opool,
        ):
            wtbc_sb = cpool.tile([96, 3, 128], bf16)
            nc.sync.dma_start(wtbc_sb[:], wtbc_t.ap().rearrange("k p m -> p k m"))
            wta_sb = cpool.tile([96, 3, 2, 128], bf16)
            nc.sync.dma_start(wta_sb[:], wta_t.ap().rearrange("k q p m -> p k q m"))
            # svb on the scalar queue, chunked, so it neither delays the
            # first imcol load (sync queue) nor makes supertile 0 wait for
            # the whole transfer
            svb_sb = cpool.tile([128, SUPER * 2 * NBLK], bf16)
            for t in range(SUPER):
                nc.scalar.dma_start(
                    svb_sb[:, 2 * NBLK * t : 2 * NBLK * (t + 1)],
                    svb_t.ap()[:, 2 * NBLK * t : 2 * NBLK * (t + 1)],
                )

            # warm-up: ~25 dummy matmuls ramp the PE clock (DVFS) while the
            # first imcol/svb DMAs stream; nothing reads the scratch bank
            warm = wpool.tile([128, 384], f32)
            for i in range(25):
                nc.tensor.matmul(
                    warm[:],
                    wtbc_sb[:, 0, :],
                    wtbc_sb[:, :, :],
                    start=(i == 0),
                    stop=(i == 24),
                )

            for b in range(NB):
                # 4 row-chunks per kh replica: supertile 0 only waits on
                # the first ~400KB instead of the whole batch's imcol
                imcol = ipool.tile([96, BAND * WP], bf16, tag="imcol")
                CH = BAND // 4
                for ch in range(4):
                    r0 = CH * ch
                    for kh in range(3):
                        nc.sync.dma_start(
                            imcol[32 * kh : 32 * kh + 32, r0 * WP : (r0 + CH) * WP],
                            xs[b, :, (kh + r0) * WP : (kh + r0 + CH) * WP],
                        )
                imv = imcol.rearrange("p (h w) -> p h w", w=WP)

                for t in range(SUPER):
                    bc = ppool.tile([128, 2 * NBLK], f32, tag="bc")
                    a2 = ppool.tile([128, NBLK], f32, tag="a2")
                    for kw in range(3):
                        for q in range(2):
                            hl = SROWS * t + 2 * q
                            rhs = imv[:, hl : hl + 2, kw : kw + IMG]
                            nc.tensor.matmul(
                                bc[:, NBLK * q : NBLK * (q + 1)],
                                wtbc_sb[:, kw, :],
                                rhs,
                                start=(kw == 0),
                                stop=(kw == 2),
                            )
                            nc.tensor.matmul(
                                a2[:],
                                wta_sb[:, kw, q, :],
                                rhs,
                                start=(kw == 0 and q == 0),
                                stop=False,
                            )

                    # blend off the PE entirely:
                    #   DVE: m1/m2 = sv_r * y_r (PSUM f32 x SB bf16 -> bf16)
                    #   Scalar: straighten a2 (q,c)x512 -> out_sb c x (q j)
                    #   DMA CCE: out_sb += m1; out_sb += m2 (SBUF->SBUF add)
                    svt = svb_sb[:, 2 * NBLK * t : 2 * NBLK * (t + 1)]
                    m1 = tpool.tile([64, 2 * NBLK], bf16, tag="m1")
                    nc.vector.tensor_tensor(
                        m1[:], bc[0:64, :], svt[0:64, :], mybir.AluOpType.mult
                    )
                    m2 = tpool.tile([64, 2 * NBLK], bf16, tag="m2")
                    nc.vector.tensor_tensor(
                        m2[:], bc[64:128, :], svt[64:128, :], mybir.AluOpType.mult
                    )
                    out_sb = opool.tile([64, 2 * NBLK], bf16, tag="out_sb")
                    for q in range(2):
                        nc.scalar.copy(
                            out_sb[:, NBLK * q : NBLK * (q + 1)],
                            a2[64 * q : 64 * q + 64, :],
                        )
                    nc.gpsimd.dma_start(
                        out_sb[:], m1[:], accum_op=mybir.AluOpType.add
                    )
                    nc.gpsimd.dma_start(
                        out_sb[:], m2[:], accum_op=mybir.AluOpType.add
                    )
                    nc.scalar.dma_start(out_r[b, t], out_sb[:])
    nc.compile()
    return nc


_CACHE = {}


def _get_bass():
    if "nc" not in _CACHE:
        _CACHE["nc"] = _build_bass()
    return _CACHE["nc"]


def _prep_shards(x, conv_w, kernel_weight):
    x = np.asarray(x, dtype=_F32)
    conv_w = np.asarray(conv_w, dtype=_F32)
    kernel_weight = np.asarray(kernel_weight, dtype=_F32)

    x_pad = np.pad(x, ((0, 0), (0, 0), (1, 1), (1, 1))).astype(_BF16)
    # w[kh, c, kw, (r, m)] from conv_w[(r m), c, kh, kw]
    wt = conv_w.transpose(2, 1, 3, 0).reshape(96, 3, RANK * C_OUT)
    wtbc = np.ascontiguousarray(
        wt[:, :, C_OUT:].reshape(96, 3, 128).transpose(1, 0, 2)
    ).astype(_BF16)  # [kw, 96, (r1|r2)]
    wta = np.zeros((3, 2, 96, 128), dtype=_F32)
    for q in range(2):
        wta[:, q, :, 64 * q : 64 * q + 64] = wt[:, :, :C_OUT].transpose(1, 0, 2)
    wta = wta.astype(_BF16)

    in_maps = []
    for i in range(N_CORES):
        h0 = BAND * i
        shard = np.ascontiguousarray(
            x_pad[:, :, h0 : h0 + ROWS_IN, :]
        ).reshape(B, C_IN, ROWS_IN * WP)
        band = kernel_weight[:, h0 : h0 + BAND, :]          # [2, 32, 256]
        # svb[64r+c, (t, q, j)] = band[r, row(t, q, j)]
        arr = band.reshape(2, SUPER, 2 * NBLK)              # [r, t, (q j)]
        svb = np.broadcast_to(
            arr[:, None, :, :], (2, C_OUT, SUPER, 2 * NBLK)
        ).reshape(128, SUPER * 2 * NBLK)
        svb = np.ascontiguousarray(svb).astype(_BF16)
        in_maps.append({"xs": shard, "wtbc": wtbc, "wta": wta, "svb": svb})
    return in_maps


def run(inputs, trace=False):
    """Run the sharded bass kernel; returns (out_full, BassKernelResults)."""
    from concourse.bass_utils import run_bass_kernel_spmd

    in_maps = _prep_shards(**inputs)
    nc = _get_bass()
    res = run_bass_kernel_spmd(
        nc, in_maps, core_ids=list(range(N_CORES)), trace=trace
    )
    out = np.empty((B, C_OUT, IMG, IMG), dtype=_F32)
    for i in range(N_CORES):
        # core layout: (b, t, c, q, r, w) -> (b, c, t, q, r, w)
        o = res.results[i]["out"].astype(_F32)
        o = o.reshape(B, SUPER, C_OUT, 2, SROWS // 2, IMG)
        o = o.transpose(0, 2, 1, 3, 4, 5).reshape(B, C_OUT, BAND, IMG)
        out[:, :, BAND * i : BAND * (i + 1), :] = o
    return out, res


def kernel(x, conv_w, kernel_weight):
    out, _ = run({"x": x, "conv_w": conv_w, "kernel_weight": kernel_weight})
    return out


# revision 31
# speedup vs baseline: 1.2792x; 1.2792x over previous
"""Trainium2 Bass kernel for nn_LRSVConv (low-rank spatially-varying conv).

Computes, for full inputs
    x            [8, 32, 256, 256]  f32
    conv_w       [192, 32, 3, 3]    f32   (192 = RANK(3) * C_OUT(64))
    kernel_weight[2, 256, 256]      f32
the reference:
    y   = conv2d(x, conv_w, stride 1, pad 1)      # [8, 192, 256, 256]
    y   = y.reshape(8, 3, 64, 256, 256)
    out = y[:,0] + kw[0]*y[:,1] + kw[1]*y[:,2]    # [8, 64, 256, 256]

Strategy: spatial (H) sharding across 8 cores - each core computes a band of
32 output rows for ALL batches, so the per-pixel blend weights (which are
batch-independent) are loaded/broadcast once per core and reused 8x.

The conv matmuls run in bf16 (inputs and weights quantized host-side; fp32
PSUM accumulate). Measured absmax rel err ~2.4e-3, inside the 2e-2 gate.
All matmuls use uniform (128,128) PE tiles (a2 weights zero-padded to
M=128) to avoid PE tile-config switches between matmuls.

Per core:
  - imcol tile [96, 32*258] bf16: 3 kh-shifted replicas of the padded input
    rows (partition dim = (kh, c_in)), padded W=258 so kw shifts are free-dim
    offsets and no edge handling is needed.
  - conv: per supertile (4 output rows = 1024 px, split into 2 blocks of
    512 px), per rank r and kw: one K=96, M=128, N=512 matmul per block,
    accumulating in PSUM; bc = ranks 1,2 [128, 1024], a2 = rank 0
    [128, 512] with psum rows = (block, c_out).
  - blend: m = bc * svb on DVE; m folded onto a2 via identity matmuls on
    TensorE (partition-fold is only possible on PE or DVE-with-PSUM);
    out = copy(a2) on Scalar (PSUM evacuation), then 2 output DMAs on the
    scalar engine's HW DGE queue (inputs load on sync's queue).
  - sv broadcast tiles prepared host-side ([128, 8192] f32: rows
    (block, c) x band pixels).
"""

import os

import numpy as np
import ml_dtypes

B, C_IN, C_OUT, RANK, IMG = 8, 32, 64, 3, 256
N_CORES = 8
BAND = IMG // N_CORES          # 32 output rows per core
WP = IMG + 2                   # padded width 258
ROWS_IN = BAND + 2             # input rows needed per band (with halo)
SUPER = 8                      # supertiles per (batch, band): 4 rows each
SROWS = BAND // SUPER          # 4 image rows per supertile
NBLK = 512                     # pixels per matmul block (2 image rows)

_F32 = np.float32
_BF16 = ml_dtypes.bfloat16

NB = int(os.environ.get("KERNEL_NB", str(B)))  # batches to process (debug knob)


def _build_bass():
    import concourse.mybir as mybir
    import concourse.tile as tile
    from concourse import bacc

    f32 = mybir.dt.float32
    f32r = mybir.dt.float32r
    bf16 = mybir.dt.bfloat16
    nc = bacc.Bacc("TRN2", target_bir_lowering=False, debug=False)

    xs_t = nc.dram_tensor("xs", (B, C_IN, ROWS_IN * WP), bf16, kind="ExternalInput")
    # wtbc[kw]: [96, (rank1|rank2)]; wta[kw, q]: [96, (w0|0) or (0|w0)]
    wtbc_t = nc.dram_tensor("wtbc", (3, 96, 128), bf16, kind="ExternalInput")
    wta_t = nc.dram_tensor("wta", (3, 2, 96, 128), bf16, kind="ExternalInput")
    # S12: rows 0:64 = sv1, rows 64:128 = sv2; cols = (supertile, block, j)
    svb_t = nc.dram_tensor("svb", (128, SUPER * 2 * NBLK), bf16, kind="ExternalInput")
    # identII[q]: cols 64q:64q+64 hold [I64; I64] (sum the two 64-row halves)
    id_t = nc.dram_tensor("ident", (2, 128, 128), bf16, kind="ExternalInput")
    # kernel-native layout: (b, t, (q c), (r w)); host untangles it
    out_t = nc.dram_tensor("out", (B, SUPER, 128, NBLK), f32, kind="ExternalOutput")

    xs = xs_t.ap()
    out_r = out_t.ap()

    with tile.TileContext(nc) as tc:
        with (
            tc.tile_pool(name="const", bufs=1) as cpool,
            tc.tile_pool(name="imcol", bufs=2) as ipool,
            tc.tile_pool(name="psum", bufs=2, space="PSUM") as ppool,
            tc.tile_pool(name="warm", bufs=1, space="PSUM") as wpool,
            tc.tile_pool(name="tmp", bufs=6) as tpool,
            tc.tile_pool(name="outp", bufs=6) as opool,
        ):
            CH = BAND // 4
            imcols = {}

            def load_imcol(b, chunks):
                imcol = imcols[b]
                for ch in chunks:
                    r0 = CH * ch
                    for kh in range(3):
                        nc.sync.dma_start(
                            imcol[32 * kh : 32 * kh + 32, r0 * WP : (r0 + CH) * WP],
                            xs[b, :, (kh + r0) * WP : (kh + r0 + CH) * WP],
                        )

            wtbc_sb = cpool.tile([96, 3, 128], bf16)
            nc.sync.dma_start(wtbc_sb[:], wtbc_t.ap().rearrange("k p m -> p k m"))
            # first chunk of batch 0 ahead of wta: the first bc matmul only
            # needs wtbc + this chunk
            imcols[0] = ipool.tile([96, BAND * WP], bf16, tag="imcol", name="imcol")
            load_imcol(0, [0])
            wta_sb = cpool.tile([96, 3, 2, 128], bf16)
            nc.sync.dma_start(wta_sb[:], wta_t.ap().rearrange("k q p m -> p k q m"))
            load_imcol(0, [1, 2, 3])
            id_sb = cpool.tile([128, 2, 128], bf16)
            nc.scalar.dma_start(id_sb[:], id_t.ap().rearrange("q p m -> p q m"))
            # svb on the scalar queue, chunked, so it neither delays the
            # first imcol load (sync queue) nor makes supertile 0 wait for
            # the whole transfer
            svb_sb = cpool.tile([128, SUPER * 2 * NBLK], bf16)
            for t in range(SUPER):
                nc.scalar.dma_start(
                    svb_sb[:, 2 * NBLK * t : 2 * NBLK * (t + 1)],
                    svb_t.ap()[:, 2 * NBLK * t : 2 * NBLK * (t + 1)],
                )

            # warm-up: ~25 dummy matmuls ramp the PE clock (DVFS) while the
            # first imcol/svb DMAs stream; nothing reads the scratch bank
            warm = wpool.tile([128, 384], f32)
            for i in range(25):
                nc.tensor.matmul(
                    warm[:],
                    wtbc_sb[:, 0, :],
                    wtbc_sb[:, :, :],
                    start=(i == 0),
                    stop=(i == 24),
                )

            # software pipeline: fold/evac/store for supertile i-1 are
            # emitted after supertile i's conv matmuls, so the PE never
            # waits on the DVE blend of the supertile it just computed
            pend = None  # (b, t, bc, a2, m)

            def flush(pend):
                pb, pt, _, pa2, pm = pend
                for q in range(2):
                    nc.tensor.matmul(
                        pa2[:],
                        id_sb[:, q, :],
                        pm[:, NBLK * q : NBLK * (q + 1)],
                        start=False,
                        stop=(q == 1),
                    )
                out_sb = opool.tile([128, NBLK], f32, tag="out_sb")
                nc.scalar.copy(out_sb[:], pa2[:])
                nc.scalar.dma_start(out_r[pb, pt], out_sb[:])

            for b in range(NB):
                # 4 row-chunks per kh replica: supertile 0 only waits on
                # the first ~400KB instead of the whole batch's imcol
                if b not in imcols:
                    imcols[b] = ipool.tile([96, BAND * WP], bf16, tag="imcol", name="imcol")
                    load_imcol(b, [0, 1, 2, 3])
                imv = imcols[b].rearrange("p (h w) -> p h w", w=WP)

                for t in range(SUPER):
                    bc = ppool.tile([128, 2 * NBLK], f32, tag="bc")
                    a2 = ppool.tile([128, NBLK], f32, tag="a2")
                    for kw in range(3):
                        for q in range(2):
                            hl = SROWS * t + 2 * q
                            rhs = imv[:, hl : hl + 2, kw : kw + IMG]
                            nc.tensor.matmul(
                                bc[:, NBLK * q : NBLK * (q + 1)],
                                wtbc_sb[:, kw, :],
                                rhs,
                                start=(kw == 0),
                                stop=(kw == 2),
                            )
                            nc.tensor.matmul(
                                a2[:],
                                wta_sb[:, kw, q, :],
                                rhs,
                                start=(kw == 0 and q == 0),
                                stop=False,
                            )

                    if pend is not None:
                        flush(pend)

                    # m = [sv1*y1 ; sv2*y2] for both blocks, one 128-row op
                    m = tpool.tile([128, 2 * NBLK], bf16, tag="m")
                    nc.vector.tensor_tensor(
                        m[:],
                        bc,
                        svb_sb[:, 2 * NBLK * t : 2 * NBLK * (t + 1)],
                        mybir.AluOpType.mult,
                    )
                    pend = (b, t, bc, a2, m)

            flush(pend)
    nc.compile()
    return nc


_CACHE = {}


def _get_bass():
    if "nc" not in _CACHE:
        _CACHE["nc"] = _build_bass()
    return _CACHE["nc"]


def _prep_shards(x, conv_w, kernel_weight):
    x = np.asarray(x, dtype=_F32)
    conv_w = np.asarray(conv_w, dtype=_F32)
    kernel_weight = np.asarray(kernel_weight, dtype=_F32)

    x_pad = np.pad(x, ((0, 0), (0, 0), (1, 1), (1, 1))).astype(_BF16)
    # w[kh, c, kw, (r, m)] from conv_w[(r m), c, kh, kw]
    wt = conv_w.transpose(2, 1, 3, 0).reshape(96, 3, RANK * C_OUT)
    wtbc = np.ascontiguousarray(
        wt[:, :, C_OUT:].reshape(96, 3, 128).transpose(1, 0, 2)
    ).astype(_BF16)  # [kw, 96, (r1|r2)]
    wta = np.zeros((3, 2, 96, 128), dtype=_F32)
    for q in range(2):
        wta[:, q, :, 64 * q : 64 * q + 64] = wt[:, :, :C_OUT].transpose(1, 0, 2)
    wta = wta.astype(_BF16)
    ident = np.zeros((2, 128, 128), dtype=_F32)
    for q in range(2):
        ident[q, 0:64, 64 * q : 64 * q + 64] = np.eye(64, dtype=_F32)
        ident[q, 64:128, 64 * q : 64 * q + 64] = np.eye(64, dtype=_F32)
    ident = ident.astype(_BF16)

    in_maps = []
    for i in range(N_CORES):
        h0 = BAND * i
        shard = np.ascontiguousarray(
            x_pad[:, :, h0 : h0 + ROWS_IN, :]
        ).reshape(B, C_IN, ROWS_IN * WP)
        band = kernel_weight[:, h0 : h0 + BAND, :]          # [2, 32, 256]
        # svb[64r+c, (t, q, j)] = band[r, row(t, q, j)]
        arr = band.reshape(2, SUPER, 2 * NBLK)              # [r, t, (q j)]
        svb = np.broadcast_to(
            arr[:, None, :, :], (2, C_OUT, SUPER, 2 * NBLK)
        ).reshape(128, SUPER * 2 * NBLK)
        svb = np.ascontiguousarray(svb).astype(_BF16)
        in_maps.append(
            {"xs": shard, "wtbc": wtbc, "wta": wta, "svb": svb, "ident": ident}
        )
    return in_maps


def run(inputs, trace=False):
    """Run the sharded bass kernel; returns (out_full, BassKernelResults)."""
    from concourse.bass_utils import run_bass_kernel_spmd

    in_maps = _prep_shards(**inputs)
    nc = _get_bass()
    res = run_bass_kernel_spmd(
        nc, in_maps, core_ids=list(range(N_CORES)), trace=trace
    )
    out = np.empty((B, C_OUT, IMG, IMG), dtype=_F32)
    for i in range(N_CORES):
        # core layout: (b, t, q, c, r, w) -> (b, c, t, q, r, w)
        o = res.results[i]["out"].reshape(B, SUPER, 2, C_OUT, SROWS // 2, IMG)
        o = o.transpose(0, 3, 1, 2, 4, 5).reshape(B, C_OUT, BAND, IMG)
        out[:, :, BAND * i : BAND * (i + 1), :] = o
    return out, res


def kernel(x, conv_w, kernel_weight):
    out, _ = run({"x": x, "conv_w": conv_w, "kernel_weight": kernel_weight})
    return out
